# revision 64
# baseline (speedup 1.0000x reference)
"""Trainium2 Bass kernel for nn_Attention_KV (dense transformer attention
with K=Q sharing and a linear positional bias), distributed over 8 cores.

Sharding: each core owns ALL 8 batches for one 128-row query octant.
The j (key) axis is rolled by -128*c per core on the host (pure layout),
so the SPMD-uniform program always finds its own query block in columns
0:128 of the on-device K^T — no per-core addressing, no shipped xq, and
pos ships exactly once across the 8 cores (i-octant slice, bf16).
Collectives are avoided entirely (~300us fixed latency each on this
fabric, measured in an earlier session).

Everything the PE touches is bf16 (f32 PSUM accumulation), which runs
1 cycle/row at any free size (f32r needs free>=256) and halves DMA +
SBUF. Attention keeps scores TRANSPOSED ([j, i]: keys on partitions):
  - dots^T lands in PSUM 4 heads at a time ([128 j, 4*128 i]); the
    pos bias is added by an identity-matmul whose rhs is pos_bias^T/c
    (pre-divided on device via wposr/c), so exp(scale=c) on the Scalar
    engine applies the dot scaling and the bias in one pass
  - attn@v: lhsT = v_ext (ones column appended -> row 64 of the result
    is the softmax denominator Z), rhs = exp(scores^T) bf16
  - normalization: Z row -> DVE reciprocal -> PE K=1 broadcast matmul
    ([1,64] ones x [1,512] rz -> [64,512] PSUM) -> DVE multiply, no
    DRAM bounce
  - output projection packs head pairs so K=128 (4 matmuls), bias via
    a K=1 ones x b_out matmul, per-batch [128 i, 512] f32 out
PSUM->SBUF copies are spread across Scalar (k^T), Pool (v, y) and DVE
(pos phase, normalize) so no single helper engine becomes critical.
b_pos (a scalar added to every score) is dropped: softmax is shift
invariant.
"""

import sys

sys.path.insert(0, "/opt/trn_rl_repo")

import numpy as np

import concourse.bacc as bacc
import concourse.bass as bass
import concourse.mybir as mybir
from concourse import tile
from concourse.bass_utils import run_bass_kernel_spmd

B, N, DIM, H, POS_DIM = 8, 1024, 512, 8, 50
D = DIM // H  # 64
NC = 8  # cores
IO = 128  # query rows per core (i-octant)
JT = N // 128  # 8 j-tiles
SCALE = float(DIM) ** -0.5

F32 = mybir.dt.float32
F32R = mybir.dt.float32r
BF16 = mybir.dt.bfloat16
FP8 = mybir.dt.float8e4
AX = mybir.AxisListType
ALU = mybir.AluOpType
ACTF = mybir.ActivationFunctionType

POS_CHUNK = 64  # i-columns of pos processed per DVE reduce


def build_program():
    nc = bacc.Bacc("TRN2", target_bir_lowering=False, debug=False)

    # ---- DRAM parameters (per-core) ----
    xT_d = nc.declare_dram_parameter("xT", [B, DIM, N], FP8, isOutput=False)
    wkvT_d = nc.declare_dram_parameter("wkvT", [DIM, 2 * DIM], BF16, isOutput=False)
    wout_d = nc.declare_dram_parameter("wout", [4, 2 * D, DIM], BF16, isOutput=False)
    bout_d = nc.declare_dram_parameter("bout", [1, DIM], F32R, isOutput=False)
    wposr_d = nc.declare_dram_parameter(
        "wposr", [128, POS_CHUNK, POS_DIM], BF16, isOutput=False
    )
    posT_d = nc.declare_dram_parameter("posT", [N, IO, POS_DIM], FP8, isOutput=False)
    ones_d = nc.declare_dram_parameter("ones", [65, 128], F32R, isOutput=False)
    ones16_d = nc.declare_dram_parameter("ones16", [128, H], BF16, isOutput=False)
    y_d = nc.declare_dram_parameter("y", [B, IO, DIM], F32, isOutput=True)

    with tile.TileContext(nc) as tc:
        with (
            tc.tile_pool(name="persist", bufs=1) as pp,
            tc.tile_pool(name="pos_in", bufs=2) as pos_pool,
            tc.tile_pool(name="exps", bufs=10) as epool,
            tc.tile_pool(name="expraw", bufs=5) as erpool,
            tc.tile_pool(name="outsb", bufs=2) as opool,
            tc.tile_pool(name="mm_ps", bufs=2, space="PSUM") as mmps,
            tc.tile_pool(name="dots_ps", bufs=3, space="PSUM") as dotsps,
            tc.tile_pool(name="up_ps", bufs=2, space="PSUM") as upps,
            tc.tile_pool(name="rz_ps", bufs=1, space="PSUM") as rzps,
        ):
            # ---- preload weights + small tensors ----
            wposr = pp.tile([128, POS_CHUNK, POS_DIM], BF16, tag="wposr")
            nc.sync.dma_start(wposr[:], wposr_d[:])
            wkvT = [
                pp.tile([128, 2 * DIM], BF16, name=f"wkvT{t}", tag=f"wkvT{t}")
                for t in range(4)
            ]
            for t in range(4):
                nc.sync.dma_start(wkvT[t][:], wkvT_d[t * 128 : (t + 1) * 128, :])
            wout = [
                pp.tile([2 * D, DIM], BF16, name=f"wout{k}", tag=f"wout{k}")
                for k in range(4)
            ]
            for k in range(4):
                nc.sync.dma_start(wout[k][:], wout_d[k, :, :])
            bout = pp.tile([1, DIM], F32R, tag="bout")
            nc.sync.dma_start(bout[:], bout_d[:])

            # row 0: lhsT for the bias matmul; row 64: lhsT for the 1/Z
            # broadcast matmul — it sits at partition 64 to match the Z
            # row's PSUM partition (engines can't shift partitions, and
            # matmul requires lhsT/rhs at the same base partition).
            onesr = pp.tile([65, 128], F32R, tag="onesr")
            nc.sync.dma_start(onesr[:], ones_d[:])

            # c = scale * sum(w_pos) on every partition (exp scale for dots)
            c_ap = pp.tile([128, 1], F32, tag="c_ap")
            nc.vector.tensor_reduce(c_ap[:], wposr[:, 0, :], axis=AX.X, op=ALU.add)
            nc.scalar.mul(c_ap[:], c_ap[:], SCALE)

            # ---- per batch state ----
            xT_sets = {
                s2: [
                    pp.tile([128, N], FP8, name=f"xT{t}_{s2}", tag=f"xT{t}_{s2}")
                    for t in range(4)
                ]
                for s2 in (0, 1)
            }
            kT_sets = {
                s2: [
                    pp.tile([128, N], BF16, name=f"kT{t}_{s2}", tag=f"kT{t}_{s2}")
                    for t in range(4)
                ]
                for s2 in (0, 1)
            }
            # odd heads' K rows shifted to partition base 0 (SBUF->SBUF
            # DMA can cross partitions; engines cannot) so every dots
            # matmul uses tile position (0, 0)
            kTodd_sets = {
                s2: [
                    pp.tile([64, N], BF16, name=f"kTo{t}_{s2}", tag=f"kTo{t}_{s2}")
                    for t in range(4)
                ]
                for s2 in (0, 1)
            }
            vext_sets = {
                s2: [
                    pp.tile(
                        [128, H, D + 1],
                        BF16,
                        name=f"vext{t}_{s2}",
                        tag=f"vext{t}_{s2}",
                    )
                    for t in range(JT)
                ]
                for s2 in (0, 1)
            }
            # normalized attn output in head-pair layout [128 (2h,d), i]
            # so the output projection runs K=128; odd heads stage at base
            # 0 (DVE cannot shift partitions) then DMA into rows 64:128
            upair_sets = {
                s2: [
                    pp.tile([128, IO], BF16, name=f"upr{k}_{s2}", tag=f"upr{k}_{s2}")
                    for k in range(4)
                ]
                for s2 in (0, 1)
            }
            usb_sets = {
                s2: [
                    pp.tile([64, IO], BF16, name=f"usb{h}_{s2}", tag=f"usb{h}_{s2}")
                    for h in range(H)
                ]
                for s2 in (0, 1)
            }
            # 1/Z staging; row 64 only (same partition as the PSUM Z row),
            # one 512-column block per head group
            rz_sets = {
                s2: pp.tile([65, 1024], F32R, name=f"rz_{s2}", tag=f"rz_{s2}")
                for s2 in (0, 1)
            }

            # vext ones columns survive across batches (the per-batch copy
            # only writes cols 0:D), so set them once at preload
            for s2 in (0, 1):
                for nt in range(JT):
                    nc.sync.dma_start(
                        vext_sets[s2][nt][:, :, D : D + 1], ones16_d[:, :]
                    )

            # posT1_sb[jt] = pos_bias^T; E4[jt] = exp(pos_bias^T) x4 along
            # free so the bias folds into softmax as es = exp(c*dots) * E
            # (no PE work for the bias — the PE is the throttled engine)
            E4 = [
                pp.tile([128, 512], BF16, name=f"E4_{j}", tag=f"E4_{j}")
                for j in range(JT)
            ]
            posT1_sb = [
                pp.tile([128, IO], BF16, name=f"posT1_{j}", tag=f"posT1_{j}")
                for j in range(JT)
            ]

            def emit_pos_jt(jt):
                    for ic in range(IO // POS_CHUNK):
                        sl = slice(ic * POS_CHUNK, (ic + 1) * POS_CHUNK)
                        pt8 = pos_pool.tile(
                            [128, POS_CHUNK, POS_DIM], FP8, name="pchunk8", tag="pchunk8"
                        )
                        nc.sync.dma_start(
                            pt8[:], posT_d[jt * 128 : (jt + 1) * 128, sl, :]
                        )
                        pt = pos_pool.tile(
                            [128, POS_CHUNK, POS_DIM], BF16, name="pchunk", tag="pchunk"
                        )
                        nc.vector.tensor_tensor(pt[:], pt8[:], wposr[:], op=ALU.mult)
                        with nc.allow_low_precision(
                            reason="pos bias flows in bf16 by design"
                        ):
                            nc.vector.tensor_reduce(
                                posT1_sb[jt][:, sl], pt[:], axis=AX.X, op=ALU.add
                            )
                    for r in range(4):
                        nc.scalar.activation(
                            E4[jt][:, r * IO : (r + 1) * IO],
                            posT1_sb[jt][:],
                            ACTF.Exp,
                        )

            def emit_pos():
                for jt in range(JT):
                    emit_pos_jt(jt)

            def emit_kv(b):
                s2 = b % 2
                xT = xT_sets[s2]
                for t in range(4):
                    nc.sync.dma_start(xT[t][:], xT_d[b, t * 128 : (t + 1) * 128, :])
                kT = kT_sets[s2]
                for t in range(4):
                    for nchunk in range(2):
                        ps = mmps.tile([128, 512], F32, name="mmtile", tag="mm")
                        for dc in range(4):
                            nc.tensor.matmul(
                                ps[:],
                                wkvT[dc][:, t * 128 : (t + 1) * 128],
                                xT[dc][:, nchunk * 512 : (nchunk + 1) * 512],
                                start=(dc == 0),
                                stop=(dc == 3),
                            )
                        nc.scalar.copy(
                            kT[t][:, nchunk * 512 : (nchunk + 1) * 512], ps[:]
                        )
                for t in range(4):
                    nc.sync.dma_start(kTodd_sets[s2][t][:], kT[t][64:128, :])
                vext = vext_sets[s2]
                for nt in range(JT):
                    ps = mmps.tile([128, 512], F32, name="mmtile", tag="mm")
                    for dc in range(4):
                        nc.tensor.matmul(
                            ps[:],
                            xT[dc][:, nt * 128 : (nt + 1) * 128],
                            wkvT[dc][:, DIM : 2 * DIM],
                            start=(dc == 0),
                            stop=(dc == 3),
                        )
                    nc.vector.tensor_copy(
                        vext[nt][:, :, 0:D],
                        ps[:].rearrange("p (h d) -> p h d", h=H),
                    )

            def emit_attn(b):
                s2 = b % 2
                kT = kT_sets[s2]
                vext = vext_sets[s2]
                usb = usb_sets[s2]
                rz = rz_sets[s2]
                for g in range(2):  # head groups of 4
                    up = upps.tile([D + 1, 512], F32, name="uptile", tag="up")
                    es_tiles = []
                    for jt in range(JT):
                        if b == 0 and g == 0:
                            # just-in-time pos for batch 0: es(jt) then only
                            # waits for pos(jt) on the DVE queue, not the
                            # whole pos phase (measured 75us PE gap)
                            emit_pos_jt(jt)
                        dots = dotsps.tile([128, 512], F32, name="dotstile", tag="dots")
                        for h4 in range(4):
                            h = 4 * g + h4
                            csl = slice(h4 * IO, (h4 + 1) * IO)
                            if h % 2 == 0:
                                src_k = kT[h // 2]
                                pr = slice(0, 64)
                            else:
                                src_k = kTodd_sets[s2][h // 2]
                                pr = slice(0, 64)
                            nc.tensor.matmul(
                                dots[:, csl],
                                src_k[pr, jt * 128 : (jt + 1) * 128],
                                src_k[pr, 0:IO],
                                start=True,
                                stop=True,
                            )
                        er = erpool.tile([128, 512], BF16, name="expR", tag="expR")
                        nc.scalar.activation(er[:], dots[:], ACTF.Exp, scale=c_ap[:])
                        es = epool.tile([128, 512], BF16, name="expS", tag="expS")
                        nc.vector.tensor_tensor(es[:], er[:], E4[jt][:], op=ALU.mult)
                        es_tiles.append(es)
                    # PSUM allows one open accumulation group per bank at a
                    # time, so each head's jt-chain must run start->stop
                    # consecutively (jt inner, head outer).
                    for h4 in range(4):
                        h = 4 * g + h4
                        csl = slice(h4 * IO, (h4 + 1) * IO)
                        for jt in range(JT):
                            nc.tensor.matmul(
                                up[:, csl],
                                vext[jt][:, h, :],
                                es_tiles[jt][:, csl],
                                start=(jt == 0),
                                stop=(jt == JT - 1),
                            )
                    # normalize: Z row -> recip -> PE broadcast -> DVE mult
                    gsl = slice(g * 512, (g + 1) * 512)
                    with nc.allow_low_precision(
                        reason="f32r is bit-identical to f32; matmul encoding only"
                    ):
                        nc.vector.reciprocal(rz[D : D + 1, gsl], up[D : D + 1, :])
                    rzb = rzps.tile([64, 512], F32, name="rzb", tag="rzb")
                    nc.tensor.matmul(
                        rzb[:],
                        onesr[D : D + 1, 0:64],
                        rz[D : D + 1, gsl],
                        start=True,
                        stop=True,
                    )
                    # DVE allows only one PSUM operand; stage 1/Z in SBUF
                    rzsb = epool.tile([64, 512], F32, name="rzsb", tag="rzsb")
                    nc.scalar.copy(rzsb[:], rzb[:])
                    for h4 in range(4):
                        h = 4 * g + h4
                        csl = slice(h4 * IO, (h4 + 1) * IO)
                        k = h // 2
                        dst = upair_sets[s2][k][0:D, :] if h % 2 == 0 else usb[h][:]
                        with nc.allow_low_precision(
                            reason="normalized attn output in bf16 by design"
                        ):
                            nc.vector.tensor_tensor(
                                dst,
                                up[0:D, csl],
                                rzsb[:, csl],
                                op=ALU.mult,
                            )
                        if h % 2 == 1:
                            nc.sync.dma_start(
                                upair_sets[s2][k][D : 2 * D, :], usb[h][:]
                            )

            def emit_final(b):
                s2 = b % 2
                upair = upair_sets[s2]
                fps = mmps.tile([128, 512], F32, name="mmtile", tag="mm")
                for k in range(4):
                    nc.tensor.matmul(
                        fps[:], upair[k][:], wout[k][:], start=(k == 0), stop=False
                    )
                nc.tensor.matmul(
                    fps[:], onesr[0:1, :], bout[:], start=False, stop=True
                )
                ot = opool.tile([128, 512], F32, name="osb", tag="osb")
                nc.scalar.copy(ot[:], fps[:])
                nc.sync.dma_start(y_d[b, :, :], ot[:])

            emit_kv(0)
            emit_kv(1)
            emit_attn(0)
            emit_final(0)
            for b in range(2, B):
                emit_kv(b)
                emit_attn(b - 1)
                emit_final(b - 1)
            emit_attn(B - 1)
            emit_final(B - 1)

    nc.compile()
    return nc


_CACHE = {}


def _get_program():
    if "nc" not in _CACHE:
        _CACHE["nc"] = build_program()
    return _CACHE["nc"]


def _host_shard(x, pos, W_kv, W_out, b_out, w_pos, b_pos):
    """Build the 8 per-core input maps (pure layout work, no math)."""
    import ml_dtypes

    bf16 = ml_dtypes.bfloat16
    fp8 = ml_dtypes.float8_e4m3
    x = np.asarray(x, dtype=np.float32)
    pos = np.asarray(pos, dtype=np.float32)
    W_kv = np.asarray(W_kv, dtype=np.float32)
    W_out = np.asarray(W_out, dtype=np.float32)
    b_out = np.asarray(b_out, dtype=np.float32)
    w_pos = np.asarray(w_pos, dtype=np.float32)

    xT = np.ascontiguousarray(x.transpose(0, 2, 1)).astype(fp8)  # (8, 512, 1024)
    wkvT = np.ascontiguousarray(W_kv.T).astype(bf16)  # (512, 1024)
    woutH = np.ascontiguousarray(W_out.T.reshape(4, 2 * D, DIM)).astype(bf16)
    boutr = b_out.reshape(1, DIM)
    wposr = np.ascontiguousarray(
        np.broadcast_to(w_pos.astype(bf16), (128, POS_CHUNK, POS_DIM))
    )
    ones_arr = np.ones((65, 128), dtype=np.float32)
    ones16_arr = np.ones((128, H), dtype=bf16)
    pos_bf = pos[0].astype(fp8)  # (1024 i, 1024 j, 50)

    in_maps = []
    for c in range(NC):
        s = c * IO
        isl = slice(s, s + IO)
        # roll x's sequence axis by -s so this core's queries are cols 0:128
        xTr = np.ascontiguousarray(
            np.concatenate([xT[:, :, s:], xT[:, :, :s]], axis=2)
        )
        pT = pos_bf[isl].transpose(1, 0, 2)  # (1024 j, 128 i, 50)
        posT = np.ascontiguousarray(np.concatenate([pT[s:], pT[:s]], axis=0))
        in_maps.append(
            {
                "xT": xTr,
                "wkvT": wkvT,
                "wout": woutH,
                "bout": boutr,
                "wposr": wposr,
                "posT": posT,
                "ones": ones_arr,
                "ones16": ones16_arr,
            }
        )
    return in_maps


def kernel(**inputs) -> np.ndarray:
    nc = _get_program()
    in_maps = _host_shard(**inputs)
    res = run_bass_kernel_spmd(nc, in_maps, list(range(NC)))
    out = np.empty((B, N, DIM), dtype=np.float32)
    for c in range(NC):
        out[:, c * IO : (c + 1) * IO, :] = res.results[c]["y"]
    return out


if __name__ == "__main__":
    import reference

    inputs = {k: np.asarray(v) for k, v in reference.setup_inputs().items()}
    expected = np.asarray(reference.reference(**inputs))
    actual = kernel(**inputs)
    err = np.abs(actual - expected).max()
    rel = err / np.abs(expected).max()
    print(f"absmax err: {err:.3e}  rel: {rel:.3e}")
